# revision 23
# baseline (speedup 1.0000x reference)
"""MoE gate routing kernel for Trainium2 (8 NeuronCores, data-parallel over tokens).

Computes, for x[8192,7168], weight[256,7168], bias[256]:
    scores = sigmoid(x @ weight.T + bias)            # [N, 256]
    group top-2 sums over 8 groups of 32 -> pick best group
    top-8 experts within best group (global indices), weights = renormalized
    sigmoid scores * 2.5
Returns (w [8192,8] f32, idx [8192,8] i32).

Strategy: shard tokens 8-way (1024/core). The host packs x and W into the
exact SBUF tile layout (k-major slabs), so every slab DMA is one contiguous
DRAM region with 7-14KB descriptors (strided 512B/1KB descriptors measured
up to 17% slower). Matmul runs as float32r (full-rate fp32, 256-wide moving
dim). x streams as 8 blocks of 128 tokens x 4 k-slabs with 16 slab buffers,
so compute starts ~7us in, the DMA queue never drains, and only the final
slab's 14 matmuls plus one DVE chain trail the last input byte. Outputs
accumulate in SBUF and leave as split DMAs near the end (per-subtile output
DMAs were blocking the Sync engine's semaphore forwarding).
"""

import os
import sys

sys.path.insert(0, "/opt/trn_rl_repo")

from concurrent.futures import ThreadPoolExecutor

import numpy as np

import concourse.bass as bass
from concourse import bacc
import concourse.mybir as mybir
from concourse.bass_utils import run_bass_kernel_spmd
from concourse.tile import TileContext

N_CORES = 8
N_TOK = 8192
TOK_PC = N_TOK // N_CORES  # 1024 tokens per core
D = 7168
E = 256
G = 8  # groups
EPG = E // G  # 32 experts per group
TOPK = 8
ROUTE_SCALE = 2.5
KC = D // 128  # 56 k-chunks
NSLAB = 4
SLAB = KC // NSLAB  # 14 k-chunks per DMA slab
TBT = 128  # tokens per block (one PSUM subtile)
TB = TOK_PC // TBT  # 8 token blocks per core

f32 = mybir.dt.float32
f32r = mybir.dt.float32r
i32 = mybir.dt.int32
u32 = mybir.dt.uint32
AX = mybir.AxisListType
OP = mybir.AluOpType
ACTF = mybir.ActivationFunctionType

_cache = {}


def _build():
    nc = bacc.Bacc(None, target_bir_lowering=False)

    # host-packed: xP[tb*NSLAB+s, p, cc, t], wP[s, p, cc, e]
    xP = nc.declare_dram_parameter(
        "xP", [TB * NSLAB * 128, SLAB * TBT], f32, isOutput=False
    )
    wP = nc.declare_dram_parameter("wP", [NSLAB * 128, SLAB * E], f32, isOutput=False)
    bias = nc.declare_dram_parameter("bias", [1, E], f32, isOutput=False)
    w_out = nc.declare_dram_parameter("w_out", [TOK_PC, TOPK], f32, isOutput=True)
    idx_out = nc.declare_dram_parameter("idx_out", [TOK_PC, TOPK], i32, isOutput=True)

    xP_v = xP.rearrange("(b p) (c t) -> b p c t", p=128, c=SLAB)  # [TB*NSLAB,128,SLAB,TBT]
    wP_v = wP.rearrange("(b p) (c e) -> b p c e", p=128, c=SLAB)  # [NSLAB,128,SLAB,E]
    w_out_v = w_out.rearrange("(s p) k -> p s k", p=128)  # [128, TB, TOPK]
    idx_out_v = idx_out.rearrange("(s p) k -> p s k", p=128)

    with TileContext(nc) as tc:
        with (
            tc.tile_pool(name="const", bufs=1) as cpool,
            tc.tile_pool(name="xbuf", bufs=18) as xpool,
            tc.tile_pool(name="sb", bufs=3) as spool,
            tc.tile_pool(name="small", bufs=3) as mpool,
            tc.tile_pool(name="out", bufs=1) as opool,
            tc.tile_pool(name="psum", bufs=8, space="PSUM") as ppool,
        ):
            wt_sb = cpool.tile([128, KC, E], f32r)
            bias_sb = cpool.tile([1, E], f32)
            nc.sync.dma_start(out=bias_sb, in_=bias[:, :])
            ones_sb = cpool.tile([1, 128], f32)
            nc.vector.memset(ones_sb, 1.0)
            # gid[p, j] = j // EPG (group id of expert j), built once
            gid_sb = cpool.tile([128, E], u32)
            nc.gpsimd.iota(
                gid_sb, pattern=[[1, G], [0, EPG]], base=0, channel_multiplier=0
            )
            gidf_sb = cpool.tile([128, E], f32)
            nc.vector.tensor_copy(out=gidf_sb, in_=gid_sb)
            wacc = opool.tile([128, TB, TOPK], f32)
            iacc = opool.tile([128, TB, TOPK], u32)

            for tb in range(TB):
                slabs = []
                for s in range(NSLAB):
                    if tb == 0:
                        c0, c1 = s * SLAB, (s + 1) * SLAB
                        nc.sync.dma_start(
                            out=wt_sb[:, c0:c1, :], in_=wP_v[s].bitcast(f32r)
                        )
                    xs = xpool.tile([128, SLAB, TBT], f32r, tag="xs")
                    nc.sync.dma_start(
                        out=xs, in_=xP_v[tb * NSLAB + s].bitcast(f32r)
                    )
                    slabs.append(xs)

                ps = ppool.tile([128, E], f32, tag="ps")
                # bias preload: ps[t, e] = 1 * bias[e]
                nc.tensor.matmul(
                    out=ps, lhsT=ones_sb, rhs=bias_sb, start=True, stop=False
                )
                for c in range(KC):
                    nc.tensor.matmul(
                        out=ps,
                        lhsT=slabs[c // SLAB][:, c % SLAB, :],
                        rhs=wt_sb[:, c, :],
                        start=False,
                        stop=(c == KC - 1),
                    )

                sig = spool.tile([128, G, EPG], f32, tag="sig")
                sig_flat = sig.rearrange("p g e -> p (g e)")
                nc.scalar.activation(out=sig_flat, in_=ps, func=ACTF.Sigmoid)

                # group top-2 sum
                m1 = mpool.tile([128, G], f32, tag="m1")
                nc.vector.tensor_reduce(out=m1, in_=sig, axis=AX.X, op=OP.max)
                scr = spool.tile([128, G, EPG], f32, tag="scr")
                nc.vector.match_replace(
                    out=scr.rearrange("p g e -> p (g e)"),
                    in_to_replace=m1,
                    in_values=sig_flat,
                    imm_value=-1e30,
                )
                gs = mpool.tile([128, G], f32, tag="gs")
                nc.vector.tensor_reduce(out=gs, in_=scr, axis=AX.X, op=OP.max)
                nc.vector.tensor_add(gs, gs, m1)  # m1 + m2

                # best group id -> multiplicative mask over all 256 experts
                g8 = mpool.tile([128, 8], f32, tag="g8")
                nc.vector.max(out=g8, in_=gs)
                gi8 = mpool.tile([128, 8], u32, tag="gi8")
                nc.vector.max_index(out=gi8, in_max=g8, in_values=gs)
                gif = mpool.tile([128, 1], f32, tag="gif")
                nc.vector.tensor_copy(out=gif, in_=gi8[:, 0:1])
                masked = spool.tile([128, G, EPG], f32, tag="masked")
                masked_flat = masked.rearrange("p g e -> p (g e)")
                # masked = (gid == best_gid) * sig, fused in one DVE op
                nc.vector.scalar_tensor_tensor(
                    out=masked_flat,
                    in0=gidf_sb,
                    scalar=gif,
                    in1=sig_flat,
                    op0=OP.is_equal,
                    op1=OP.mult,
                )

                vals8 = mpool.tile([128, TOPK], f32, tag="vals8")
                nc.vector.max(out=vals8, in_=masked_flat)
                nc.vector.max_index(
                    out=iacc[:, tb, :], in_max=vals8, in_values=masked_flat
                )

                ssum = mpool.tile([128, 1], f32, tag="ssum")
                nc.vector.tensor_reduce(out=ssum, in_=vals8, axis=AX.X, op=OP.add)
                rcp = mpool.tile([128, 1], f32, tag="rcp")
                nc.vector.reciprocal(out=rcp, in_=ssum)
                nc.vector.tensor_scalar(
                    wacc[:, tb, :], vals8, rcp, ROUTE_SCALE, op0=OP.mult, op1=OP.mult
                )

            # outputs for blocks 0-6 leave while the last block computes; only
            # the last block's 8KB trails the final chain. idx goes out on the
            # gpsimd queue so the two DMAs issue concurrently.
            NE = TB - 1
            nc.sync.dma_start(out=w_out_v[:, :NE, :], in_=wacc[:, :NE, :])
            nc.gpsimd.dma_start(
                out=idx_out_v[:, :NE, :], in_=iacc[:, :NE, :].bitcast(i32)
            )
            nc.sync.dma_start(out=w_out_v[:, NE:, :], in_=wacc[:, NE:, :])
            nc.gpsimd.dma_start(
                out=idx_out_v[:, NE:, :], in_=iacc[:, NE:, :].bitcast(i32)
            )
    nc.compile()
    return nc


def kernel(x, weight, bias):
    x = np.ascontiguousarray(x, dtype=np.float32)
    weight = np.ascontiguousarray(weight, dtype=np.float32)
    bias = np.ascontiguousarray(bias, dtype=np.float32).reshape(1, E)

    if "nc" not in _cache:
        _cache["nc"] = _build()
    nc = _cache["nc"]

    # wP[s, p, cc, e] = weight.T[(s*SLAB+cc)*128+p, e]
    wPh = np.ascontiguousarray(
        weight.T.reshape(NSLAB, SLAB, 128, E).transpose(0, 2, 1, 3)
    ).reshape(NSLAB * 128, SLAB * E)

    def shard(c):
        # xP[tb, s, p, cc, t] = x[c*1024 + tb*TBT + t, (s*SLAB+cc)*128+p]
        xs = x[c * TOK_PC : (c + 1) * TOK_PC]
        return np.ascontiguousarray(
            xs.reshape(TB, TBT, NSLAB, SLAB, 128).transpose(0, 2, 4, 3, 1)
        ).reshape(TB * NSLAB * 128, SLAB * TBT)

    with ThreadPoolExecutor(N_CORES) as ex:
        xP_shards = list(ex.map(shard, range(N_CORES)))

    in_maps = [
        {"xP": xP_shards[c], "wP": wPh, "bias": bias} for c in range(N_CORES)
    ]
    trace = bool(os.environ.get("GATE_TRACE"))
    kres = run_bass_kernel_spmd(
        nc,
        in_maps,
        list(range(N_CORES)),
        trace=trace,
        tmpdir=os.environ.get("GATE_TRACE_DIR") if trace else None,
    )
    _cache["last_results"] = kres
    res = kres.results
    w = np.concatenate([res[c]["w_out"] for c in range(N_CORES)], axis=0)
    idx = np.concatenate([res[c]["idx_out"] for c in range(N_CORES)], axis=0)
    return w, idx.astype(np.int32)
